# revision 2
# baseline (speedup 1.0000x reference)
"""EqualizedOddsLoss on 8 TRN2 NeuronCores — v3 (fused 4x histogram).

Per core: w = 4*gid + lab + sign(pred) in bf16 (codes 4g+c, c in {0,1,2};
'neither' maps to 4g-1, never binned).  Counts:
  - groups 0..5 (15 of 18 codes... 18 bins): DVE tensor_scalar
    (op0=is_equal, op1=add) with accum_out — runs in the DVE 4x perf mode.
  - groups 6,7 (6 bins): ACT cumulative cuts C_k = count(w >= k) via
    Sign(w - k + 0.5) + accum (7 cuts, k=24..30), differenced on host.
Prep: ACT does sign(pred) and 4*gid (i32->bf16 with scale); DVE casts lab
and does the two bf16 adds at 2x.  Host sums partials in f64.
"""

import numpy as np

import concourse.bass as bass
import concourse.bacc as bacc
import concourse.mybir as mybir
import concourse.tile as tile
from concourse.bass_utils import run_bass_kernel_spmd

B = 16777216
G = 8
EPS = 1e-08
WEIGHT = 1.0
N_CORES = 8
N_PER_CORE = B // N_CORES          # 2,097,152
P = 128
F = 2048
T = N_PER_CORE // (P * F)          # 8 tiles
ROUND_END = (2, 5, 8)              # bin rounds after these tile counts
NR = len(ROUND_END)

DVE_CODES = [4 * g + c for g in range(6) for c in (0, 1, 2)]   # 18 bins
ACT_CUTS = [24, 25, 26, 27, 28, 29, 30]                        # 7 cuts
NB_D = len(DVE_CODES)
NB_A = len(ACT_CUTS)
NBT = NB_D + NB_A

_CACHE = {}


def _build():
    nc = bacc.Bacc("TRN2", target_bir_lowering=False, debug=False)
    f32 = mybir.dt.float32
    bf16 = mybir.dt.bfloat16
    i32 = mybir.dt.int32

    pred_ext = nc.declare_dram_parameter("predictions", [N_PER_CORE, 1], f32, isOutput=False)
    lab_ext = nc.declare_dram_parameter("labels", [N_PER_CORE, 1], f32, isOutput=False)
    gid_ext = nc.declare_dram_parameter("protected_attributes", [N_PER_CORE, 1], i32, isOutput=False)
    out_ext = nc.declare_dram_parameter("out", [P, NR * NBT], f32, isOutput=True)

    pred_v = pred_ext[:, :].rearrange("(t p f) o -> t p (f o)", t=T, p=P, f=F)
    lab_v = lab_ext[:, :].rearrange("(t p f) o -> t p (f o)", t=T, p=P, f=F)
    gid_v = gid_ext[:, :].rearrange("(t p f) o -> t p (f o)", t=T, p=P, f=F)

    with tile.TileContext(nc) as tc:
        with (
            tc.tile_pool(name="io", bufs=2) as io_pool,
            tc.tile_pool(name="work", bufs=2) as work_pool,
            tc.tile_pool(name="junk", bufs=1) as junk_pool,
            tc.tile_pool(name="wbuf", bufs=1) as w_pool,
            tc.tile_pool(name="accp", bufs=1) as acc_pool,
        ):
            w = w_pool.tile([P, T * F], bf16)
            acc = acc_pool.tile([P, NR * NBT], f32)
            bias_t = acc_pool.tile([P, NB_A], f32)
            for j, k in enumerate(ACT_CUTS):
                nc.vector.memset(bias_t[:, j : j + 1], -float(k) + 0.5)

            prev = 0
            r = 0
            for t in range(T):
                pred = io_pool.tile([P, F], f32, tag="pred")
                lab = io_pool.tile([P, F], f32, tag="lab")
                gid = io_pool.tile([P, F], i32, tag="gid")
                nc.sync.dma_start(pred[:], pred_v[t, :, :])
                nc.sync.dma_start(lab[:], lab_v[t, :, :])
                nc.sync.dma_start(gid[:], gid_v[t, :, :])

                sgn = work_pool.tile([P, F], bf16, tag="sgn")
                gid4 = work_pool.tile([P, F], bf16, tag="gid4")
                labb = work_pool.tile([P, F], bf16, tag="labb")
                lb = work_pool.tile([P, F], bf16, tag="lb")

                nc.scalar.activation(sgn[:], pred[:], mybir.ActivationFunctionType.Sign)
                nc.scalar.activation(gid4[:], gid[:], mybir.ActivationFunctionType.Copy, scale=4.0)
                nc.vector.tensor_copy(labb[:], lab[:])
                nc.vector.tensor_tensor(lb[:], labb[:], sgn[:], op=mybir.AluOpType.add)
                nc.vector.tensor_tensor(
                    w[:, t * F : (t + 1) * F], lb[:], gid4[:], op=mybir.AluOpType.add
                )

                if t + 1 in ROUND_END:
                    lo, hi = prev * F, (t + 1) * F
                    wspan = w[:, lo:hi]
                    for j, code in enumerate(DVE_CODES):
                        junk = junk_pool.tile([P, 3 * F], bf16, tag=f"junk{j % 2}")
                        nc.vector.tensor_scalar(
                            junk[:, : hi - lo], wspan, float(code), 0.0,
                            op0=mybir.AluOpType.is_equal,
                            op1=mybir.AluOpType.add,
                            accum_out=acc[:, r * NBT + j : r * NBT + j + 1],
                        )
                    for j in range(NB_A):
                        junka = junk_pool.tile([P, 3 * F], bf16, tag=f"junka{j % 2}")
                        nc.scalar.activation(
                            junka[:, : hi - lo], wspan,
                            mybir.ActivationFunctionType.Sign,
                            bias=bias_t[:, j : j + 1],
                            accum_out=acc[:, r * NBT + NB_D + j : r * NBT + NB_D + j + 1],
                        )
                    prev = t + 1
                    r += 1
            nc.sync.dma_start(out_ext[:, :], acc[:])
    nc.compile()
    return nc


def _get_nc():
    if "nc" not in _CACHE:
        _CACHE["nc"] = _build()
    return _CACHE["nc"]


def make_in_maps(pred, lab, gid):
    in_maps = []
    for c in range(N_CORES):
        s = slice(c * N_PER_CORE, (c + 1) * N_PER_CORE)
        in_maps.append(
            {
                "predictions": pred[s],
                "labels": lab[s],
                "protected_attributes": gid[s],
            }
        )
    return in_maps


def finish(outs):
    round_elems = []
    prev = 0
    for e in ROUND_END:
        round_elems.append((e - prev) * F)
        prev = e

    dve = np.zeros(NB_D, dtype=np.float64)
    cuts = np.zeros(NB_A, dtype=np.float64)
    for c in range(N_CORES):
        a = np.asarray(outs[c]["out"], dtype=np.float64)  # [P, NR*NBT]
        s = a.sum(axis=0).reshape(NR, NBT)
        dve += s[:, :NB_D].sum(axis=0)
        for r in range(NR):
            # ACT accum = sum sign(w-k+0.5) = 2*C_k - Ncnt (per partition row)
            cuts += (s[r, NB_D:] + P * round_elems[r]) / 2.0

    n = np.zeros((G, 3), dtype=np.float64)   # [g, c] c in {0,1,2}
    for j, code in enumerate(DVE_CODES):
        n[code // 4, code % 4] += dve[j]
    C = {k: cuts[j] for j, k in enumerate(ACT_CUTS)}
    n[6, 0] = C[24] - C[25]
    n[6, 1] = C[25] - C[26]
    n[6, 2] = C[26] - C[27]
    n[7, 0] = C[28] - C[29]
    n[7, 1] = C[29] - C[30]
    n[7, 2] = C[30]

    tp = n[:, 2]
    pos = n[:, 0] + n[:, 2]
    fp = n[:, 1]
    neg = B - pos
    tpr = tp / (pos + EPS)
    fpr = fp / (neg + EPS)
    d = np.abs(tpr[:, None] - tpr[None, :]) + np.abs(fpr[:, None] - fpr[None, :])
    iu = np.triu(np.ones((G, G), dtype=bool), k=1)
    return np.float32(WEIGHT * np.sum(np.where(iu, d, 0.0)))


def kernel(predictions, labels, protected_attributes, num_groups):
    num_groups = int(num_groups)
    assert num_groups == G and predictions.shape[0] == B

    pred = np.ascontiguousarray(predictions, dtype=np.float32)
    lab = np.ascontiguousarray(labels, dtype=np.float32)
    gid = np.ascontiguousarray(protected_attributes, dtype=np.int32)

    nc = _get_nc()
    res = run_bass_kernel_spmd(nc, make_in_maps(pred, lab, gid),
                               core_ids=list(range(N_CORES)))
    outs = res.results if hasattr(res, "results") else res
    return finish(outs)
